# revision 42
# baseline (speedup 1.0000x reference)
"""BERT parallel self-attention on 8 Trainium2 NeuronCores (Bass/Tile).

Self-contained: kernel(**inputs) takes the FULL inputs
  hidden_states [2, 4096, 768] f32, attention_mask [2, 1, 1, 4096] f32,
  W_qkv [768, 2304] f32, b_qkv [2304] f32
and returns the FULL context output [2, 4096, 768] f32.

Sharding (Megatron-style tensor-parallel over heads + data-parallel over
batch): core c handles batch c//4, heads 3*(c%4)..3*(c%4)+2. Each core runs
an identical SPMD program on its shard; host gathers the 8 outputs.

Per-core device program:
  1. hidden arrives host-pre-transposed as [768, S] bf16; plain DMA into
     hT (h on partitions), chunked so QKV matmuls start early.
  2. mixed_T[f, t] = W^T hsT (PE, bf16). Host packs W columns
     [Q0|Q1|K0|K1|Q2|K2|V0|V1|V2] so f-block 0 -> Q_T of heads 0,1 stacked at
     partitions 0-63/64-127 (row-pair layout for the 128x128 PE array),
     f-block 1 -> K_T likewise, f-block 2 -> head 2 (duplicated to both
     halves). V is computed in natural [t, f] orientation with an appended
     ones column (softmax denominator rides the ctx matmul).
  3. attention per (q-chunk, t-block): scores_T[t, q] via two row-packed
     K=64 matmuls; exp alternates between ScalarE (exact LUT exp, 5 of 8
     steps; scale=1/8 folds the 1/sqrt(sqrt(hn))^2 norm, bias=mask[t]) and
     VectorE (3 of 8 steps: Schraudolph fast-exp — one fused affine +
     round-to-nearest int16 convert whose bits are read back as bf16;
     ~+-3% prob ripple, ~1.5e-2 absmax-rel on the output). This splits the
     softmax, the throughput bottleneck, across two engines while keeping
     one producer + one consumer per tile (finer splits measured slower
     due to semaphore density). ctx_T[65, q] += [V|1]^T expS in PSUM.
  4. per (head, q-chunk): PE-transpose 128-blocks of [ctx_T|Z] -> [q, 65],
     reciprocal of Z column, scale, store [q, 192] f32. Postprocess is
     drained in small pieces between attention steps to avoid DVE bursts.
"""

from contextlib import ExitStack

import ml_dtypes
import numpy as np

import concourse.bass as bass
import concourse.mybir as mybir
import concourse.tile as tile
from concourse import bacc
from concourse.bass import ts
from concourse.bass_utils import run_bass_kernel_spmd
from concourse.masks import make_identity

F32 = mybir.dt.float32
BF16 = mybir.dt.bfloat16
I16 = mybir.dt.int16
EXP = mybir.ActivationFunctionType.Exp

# Schraudolph fast-exp constants (DVE path): exp(s/8 + m) is approximated by
# computing I = round(s*SCH_A + m*SCH_M + SCH_B) as int16 and reinterpreting
# the bits as bf16 (sign|8exp|7mant). Max prob ripple ~±3.3%, which washes
# out to ~2e-3 absmax-relative error in the context output.
_LOG2E = 1.4426950408889634
SCH_A = 16.0 * _LOG2E               # multiplies the raw score (exp(s/8))
SCH_M = 128.0 * _LOG2E              # multiplies the additive mask
SCH_B = 128.0 * (127.0 - 0.0575327) # exponent bias minus zero-mean shift

P = 128
HH = 768          # hidden size
HB = HH // P      # 6 h-blocks
NHEAD = 3         # heads per core
HN = 64
FQKV = 576        # packed feature columns per core
QCHUNK = 512
B, S, H = 2, 4096, 768
N_CORES = 8


def _build(nc: bass.Bass, S: int = S):
    TB = S // P               # token blocks
    QC = S // QCHUNK          # q chunks
    assert QC % 2 == 0

    hs_d = nc.dram_tensor("hs", [HH, S], BF16, kind="ExternalInput").ap()
    w_d = nc.dram_tensor("w", [HH, FQKV], F32, kind="ExternalInput").ap()
    b_d = nc.dram_tensor("b", [640, 1], F32, kind="ExternalInput").ap()
    bflat_d = nc.dram_tensor("bflat", [1, 640], F32, kind="ExternalInput").ap()
    mask_d = nc.dram_tensor("mask", [S, 1], F32, kind="ExternalInput").ap()
    out_d = nc.dram_tensor("out", [S, NHEAD * HN], F32, kind="ExternalOutput").ap()

    with tile.TileContext(nc) as tc, ExitStack() as st_p:
        pool_p = st_p.enter_context(tc.tile_pool(name="persist", bufs=1))

        hT = pool_p.tile([P, HB, S], BF16, tag="hT")
        QT01 = pool_p.tile([P, S], BF16, tag="QT01")
        KT01 = pool_p.tile([P, S], BF16, tag="KT01")
        QT2 = pool_p.tile([P, S], BF16, tag="QT2")
        KT2 = pool_p.tile([P, S], BF16, tag="KT2")
        VZ = pool_p.tile([P, TB, NHEAD, HN + 1], BF16, tag="VZ")
        outsb = pool_p.tile([P, TB, NHEAD * HN], F32, tag="outsb")
        wb = pool_p.tile([P, HB, FQKV], BF16, tag="wb")
        btile = pool_p.tile([P, 5], F32, tag="btile")
        masks = pool_p.tile([P, TB], F32, tag="masks")
        masks2 = pool_p.tile([P, TB], F32, tag="masks2")
        ident = pool_p.tile([P, P], F32, tag="ident")
        identb = pool_p.tile([P, P], BF16, tag="identb")

        make_identity(nc, ident[:])
        nc.vector.tensor_copy(identb[:], ident[:])
        nc.vector.memset(VZ[:, :, :, HN : HN + 1], 1.0)


        # ---- phase 1+2: load/cast/transpose hidden; QKV projection ----
        with ExitStack() as st_12:
            pool_ld = st_12.enter_context(tc.tile_pool(name="ld", bufs=3))
            pool_qkps = st_12.enter_context(
                tc.tile_pool(name="qkps", bufs=2, space="PSUM")
            )
            pool_vps = st_12.enter_context(
                tc.tile_pool(name="vps", bufs=2, space="PSUM")
            )

            wf32 = pool_ld.tile([P, HB, FQKV], F32, tag="wf32")
            for hb in range(HB):
                nc.scalar.dma_start(out=wf32[:, hb, :], in_=w_d[ts(hb, P), :])
            nc.vector.tensor_copy(wb[:], wf32[:])

            # small loads on the scalar HWDGE queue (sync queue is for hidden)
            for fb in range(5):
                nc.scalar.dma_start(out=btile[:, fb : fb + 1], in_=b_d[ts(fb, P), :])
            for tb in range(TB):
                nc.scalar.dma_start(
                    out=masks[:, tb : tb + 1], in_=mask_d[ts(tb, P), :]
                )
            # per-token bias for the DVE fast-exp path
            nc.vector.tensor_scalar(
                masks2[:], masks[:], SCH_M, SCH_B,
                op0=mybir.AluOpType.mult, op1=mybir.AluOpType.add,
            )

            # b_qkv is zeros by the problem spec, so the V bias drops out:
            # the two K=64 contraction halves below sum directly into VZ.

            # hidden arrives pre-transposed from the host ([HH, S] bf16), so
            # hT fills with plain contiguous DMAs, chunked in token-quarters
            # so early QKV/V matmuls can start after ~1/4 of the transfer.
            # (Striping across a second queue was measured slower.)
            SQ = S // 4
            for quar in range(4):
                for hb in range(HB):
                    nc.sync.dma_start(
                        out=hT[:, hb, ts(quar, SQ)],
                        in_=hs_d[ts(hb, P), ts(quar, SQ)],
                    )

            # All QKV matmuls split their K=128 h-block contraction into two
            # K=64 row-tiled halves (partitions 0:64 / 64:128). The halves
            # run concurrently on different PE row groups, and each half's
            # LDWEIGHTS hides under the other half's matmul instead of
            # serializing against an all-row matmul. The halves accumulate in
            # separate PSUM banks and are summed during the PSUM->SBUF move.
            TPQ = QCHUNK // P  # token blocks per chunk
            for tq in range(S // QCHUNK):
                for tbl in range(TPQ):
                    tb = tq * TPQ + tbl
                    # V natural orientation: lhsT = hT blocks, rhs = W_v cols
                    vvl = pool_vps.tile([P, NHEAD, HN], F32, tag="vvl")
                    vvh = pool_vps.tile([P, NHEAD, HN], F32, tag="vvh")
                    for hb in range(HB):
                        nc.tensor.matmul(
                            vvl[:],
                            hT[0:HN, hb, ts(tb, P)],
                            wb[0:HN, hb, 384:576],
                            start=(hb == 0),
                            stop=(hb == HB - 1),
                        )
                        nc.tensor.matmul(
                            vvh[:],
                            hT[HN:P, hb, ts(tb, P)],
                            wb[HN:P, hb, 384:576],
                            start=(hb == 0),
                            stop=(hb == HB - 1),
                        )
                    # one-PSUM-operand rule: lo half lands in SBUF first,
                    # then the hi half accumulates onto it
                    nc.vector.tensor_copy(VZ[:, tb, :, 0:HN], vvl[:])
                    nc.vector.tensor_tensor(
                        VZ[:, tb, :, 0:HN], vvh[:], VZ[:, tb, :, 0:HN],
                        op=mybir.AluOpType.add,
                    )

                # mixed_T f-blocks (Q0Q1, K0K1, Q2K2) for this token chunk
                for fb in range(3):
                    mml = pool_qkps.tile([P, QCHUNK], F32, tag="mml")
                    mmh = pool_qkps.tile([P, QCHUNK], F32, tag="mmh")
                    for hb in range(HB):
                        nc.tensor.matmul(
                            mml[:],
                            wb[0:HN, hb, ts(fb, P)],
                            hT[0:HN, hb, ts(tq, QCHUNK)],
                            start=(hb == 0),
                            stop=(hb == HB - 1),
                        )
                        nc.tensor.matmul(
                            mmh[:],
                            wb[HN:P, hb, ts(fb, P)],
                            hT[HN:P, hb, ts(tq, QCHUNK)],
                            start=(hb == 0),
                            stop=(hb == HB - 1),
                        )
                    dst = ts(tq, QCHUNK)
                    A = mybir.AluOpType.add

                    def lo_hi(out_ap, lo_ap, hi_ap, bias_ap):
                        nc.vector.tensor_scalar_add(out_ap, lo_ap, bias_ap)
                        nc.vector.tensor_tensor(out_ap, hi_ap, out_ap, op=A)

                    if fb == 0:
                        lo_hi(QT01[:, dst], mml[:], mmh[:], btile[:, 0:1])
                    elif fb == 1:
                        lo_hi(KT01[:, dst], mml[:], mmh[:], btile[:, 1:2])
                    else:
                        lo_hi(QT2[0:HN, dst], mml[0:HN, :], mmh[0:HN, :],
                              btile[0:HN, 2:3])
                        lo_hi(KT2[HN:P, dst], mml[HN:P, :], mmh[HN:P, :],
                              btile[HN:P, 2:3])
            # duplicate head-2 Q/K to the other partition half
            nc.sync.dma_start(out=QT2[HN:P, :], in_=QT2[0:HN, :])
            nc.sync.dma_start(out=KT2[0:HN, :], in_=KT2[HN:P, :])

        # ---- phase 3: attention ----
        with ExitStack() as st_3:
            # sc 2 bufs + ct 2 bufs fills PSUM; 3-buf sc with single-buf ct
            # was measured slower (boundary stalls + transpose-scratch
            # contention), as was splitting each step's exp across engines.
            pool_sc = st_3.enter_context(tc.tile_pool(name="sc", bufs=2, space="PSUM"))
            pool_ct = st_3.enter_context(tc.tile_pool(name="ct", bufs=2, space="PSUM"))
            pool_tp = pool_ct  # transpose psum rides the just-freed ct slot
            pool_es = st_3.enter_context(tc.tile_pool(name="es", bufs=4))
            pool_cts = st_3.enter_context(tc.tile_pool(name="cts", bufs=2))
            pool_rz = st_3.enter_context(tc.tile_pool(name="rz", bufs=2))

            pend_q = []

            def emit_postprocess(ct, head, qc):
                """Queue postprocess for ct ([65,512] psum = [ctx_T ; Z]) as
                small pieces, drained one per few steps so the DVE never gets
                a burst that backs up the exp pipeline."""
                cell = {}

                def p0():
                    cell["cts"] = pool_cts.tile(
                        [HN + 1, QCHUNK], F32, tag="cts", name="cts"
                    )
                    nc.vector.tensor_copy(cell["cts"][:], ct[:])

                def pj(j):
                    def f():
                        cts = cell["cts"]
                        tp = pool_tp.tile([P, HN + 1], F32, tag="ctA", name="tp")
                        nc.tensor.transpose(
                            tp[:], cts[:, ts(j, P)], ident[0 : HN + 1, 0 : HN + 1]
                        )
                        rz = pool_rz.tile([P, 1], F32, tag="rz", name="rz")
                        nc.vector.reciprocal(rz[:], tp[:, HN : HN + 1])
                        tb_out = qc * (QCHUNK // P) + j
                        nc.vector.tensor_scalar_mul(
                            outsb[:, tb_out, ts(head, HN)], tp[:, 0:HN], rz[:]
                        )

                    return f

                pend_q.append(p0)
                pend_q.extend(pj(j) for j in range(QCHUNK // P))

            def attn_step(tb, kA, qA, kB, qB, ctA, ctB, vA, vB, first, last,
                          use_dve=False):
                """One t-block: row-packed scores pair, exp, two ctx matmuls.

                Whole-step exp engine alternation (use_dve): ScalarE runs exact
                exp; VectorE runs Schraudolph fast-exp (affine + int16 convert
                whose bits are bf16). One producer + one consumer per tile
                keeps the semaphore graph sparse — measured faster than any
                finer-grained split of the softmax between the two engines.
                """
                sc = pool_sc.tile([P, 2, QCHUNK], F32, tag="sc")
                nc.tensor.matmul(sc[:, 0, :], kA, qA, start=True, stop=True)
                nc.tensor.matmul(sc[:, 1, :], kB, qB, start=True, stop=True)
                es = pool_es.tile([P, 2, QCHUNK], BF16, tag="es")
                if use_dve:
                    nc.vector.tensor_scalar(
                        es[:].bitcast(I16), sc[:], SCH_A,
                        masks2[:, tb : tb + 1],
                        op0=mybir.AluOpType.mult, op1=mybir.AluOpType.add,
                    )
                else:
                    nc.scalar.activation(
                        es[:], sc[:], EXP,
                        bias=masks[:, tb : tb + 1], scale=0.125,
                    )
                nc.tensor.matmul(
                    ctA[:], vA, es[:, 0, :], start=first, stop=last,
                    skip_group_check=True,
                )
                nc.tensor.matmul(
                    ctB[:], vB, es[:, 1, :], start=first, stop=last,
                    skip_group_check=True,
                )

            # (pair-half A args, pair-half B args) per phase step; postprocess
            # of the previous step is deferred past the first few attn_steps
            # of the next so the PE queue never stalls ACT at qc boundaries.
            steps = []
            for qc in range(QC):  # heads 0,1 (partition-paired)
                steps.append((QT01, KT01, (0, qc), (1, qc), 0, 1))
            for qcp in range(QC // 2):  # head 2 (self-paired across q-chunks)
                steps.append((QT2, KT2, (2, 2 * qcp), (2, 2 * qcp + 1), 2, 2))

            for QT, KT, (hA, qcA), (hB, qcB), hvA, hvB in steps:
                ctA = pool_ct.tile([HN + 1, QCHUNK], F32, tag="ctA")
                ctB = pool_ct.tile([HN + 1, QCHUNK], F32, tag="ctB")
                for tb in range(TB):
                    attn_step(
                        tb,
                        KT[0:HN, ts(tb, P)], QT[0:HN, ts(qcA, QCHUNK)],
                        KT[HN:P, ts(tb, P)], QT[HN:P, ts(qcB, QCHUNK)],
                        ctA, ctB,
                        VZ[:, tb, hvA, :], VZ[:, tb, hvB, :],
                        tb == 0, tb == TB - 1,
                        use_dve=(tb % 8) in (1, 3, 6),
                    )
                    if pend_q and 2 <= tb and tb % 3 == 2:
                        pend_q.pop(0)()
                emit_postprocess(ctA, hA, qcA)
                emit_postprocess(ctB, hB, qcB)
            while pend_q:
                pend_q.pop(0)()

            for tb in range(TB):
                nc.sync.dma_start(out=out_d[ts(tb, P), :], in_=outsb[:, tb, :])


_NC_CACHE = None


def _get_nc():
    global _NC_CACHE
    if _NC_CACHE is None:
        nc = bacc.Bacc(
            "TRN2", target_bir_lowering=False, debug=False, num_devices=N_CORES
        )
        _build(nc)
        nc.compile()
        _NC_CACHE = nc
    return _NC_CACHE


def _shard_inputs(hidden_states, attention_mask, W_qkv, b_qkv):
    in_maps = []
    for c in range(N_CORES):
        b, hg = c // 4, c % 4
        h0 = 3 * hg
        order = [(0, h0), (0, h0 + 1), (768, h0), (768, h0 + 1),
                 (0, h0 + 2), (768, h0 + 2),
                 (1536, h0), (1536, h0 + 1), (1536, h0 + 2)]
        cols = np.concatenate(
            [np.arange(off + h * HN, off + (h + 1) * HN) for off, h in order]
        )
        w = np.ascontiguousarray(W_qkv[:, cols], dtype=np.float32)
        bv = np.zeros(640, dtype=np.float32)
        bv[:FQKV] = b_qkv[cols]
        in_maps.append(
            {
                "hs": np.ascontiguousarray(hidden_states[b].T).astype(
                    ml_dtypes.bfloat16
                ),
                "w": w,
                "b": bv[:, None].copy(),
                "bflat": bv[None, :].copy(),
                "mask": np.ascontiguousarray(
                    attention_mask[b, 0, 0, :, None], dtype=np.float32
                ),
            }
        )
    return in_maps


def _unshard(results):
    out = np.empty((B, S, H), dtype=np.float32)
    for c, r in enumerate(results):
        b, hg = c // 4, c % 4
        out[b, :, hg * 192 : (hg + 1) * 192] = r["out"]
    return out


def kernel(hidden_states, attention_mask, W_qkv, b_qkv, _trace=False, _tmpdir=None):
    nc = _get_nc()
    in_maps = _shard_inputs(
        np.asarray(hidden_states), np.asarray(attention_mask),
        np.asarray(W_qkv), np.asarray(b_qkv),
    )
    res = run_bass_kernel_spmd(
        nc, in_maps, core_ids=list(range(N_CORES)), trace=_trace, tmpdir=_tmpdir
    )
    out = _unshard(res.results)
    if _trace:
        kernel.last_exec_time_ns = res.exec_time_ns
        kernel.last_results = res
    return out



# revision 48
# speedup vs baseline: 1.0677x; 1.0677x over previous
"""BERT parallel self-attention on 8 Trainium2 NeuronCores (Bass/Tile).

Self-contained: kernel(**inputs) takes the FULL inputs
  hidden_states [2, 4096, 768] f32, attention_mask [2, 1, 1, 4096] f32,
  W_qkv [768, 2304] f32, b_qkv [2304] f32
and returns the FULL context output [2, 4096, 768] f32.

Sharding (Megatron-style tensor-parallel over heads + data-parallel over
batch): core c handles batch c//4, heads 3*(c%4)..3*(c%4)+2. Each core runs
an identical SPMD program on its shard; host gathers the 8 outputs.

Per-core device program:
  1. hidden arrives host-pre-transposed as [768, S] bf16; plain DMA into
     hT (h on partitions), chunked so QKV matmuls start early.
  2. mixed_T[f, t] = W^T hsT (PE, bf16). Host packs W columns
     [Q0|Q1|K0|K1|Q2|K2|V0|V1|V2] so f-block 0 -> Q_T of heads 0,1 stacked at
     partitions 0-63/64-127 (row-pair layout for the 128x128 PE array),
     f-block 1 -> K_T likewise, f-block 2 -> head 2 (duplicated to both
     halves). V is computed in natural [t, f] orientation with an appended
     ones column (softmax denominator rides the ctx matmul).
  3. attention per (q-chunk, t-block): scores_T[t, q] via two row-packed
     K=64 matmuls; exp alternates between ScalarE (exact LUT exp, 5 of 8
     steps; scale=1/8 folds the 1/sqrt(sqrt(hn))^2 norm, bias=mask[t]) and
     VectorE (3 of 8 steps: Schraudolph fast-exp — one fused affine +
     round-to-nearest int16 convert whose bits are read back as bf16;
     ~+-3% prob ripple, ~1.5e-2 absmax-rel on the output). This splits the
     softmax, the throughput bottleneck, across two engines while keeping
     one producer + one consumer per tile (finer splits measured slower
     due to semaphore density). ctx_T[65, q] += [V|1]^T expS in PSUM.
  4. per (head, q-chunk): PE-transpose 128-blocks of [ctx_T|Z] -> [q, 65],
     reciprocal of Z column, scale, store [q, 192] f32. Postprocess is
     drained in small pieces between attention steps to avoid DVE bursts.
"""

from contextlib import ExitStack

import ml_dtypes
import numpy as np

import concourse.bass as bass
import concourse.mybir as mybir
import concourse.tile as tile
from concourse import bacc
from concourse.bass import ts
from concourse.bass_utils import run_bass_kernel_spmd
from concourse.masks import make_identity

F32 = mybir.dt.float32
BF16 = mybir.dt.bfloat16
I16 = mybir.dt.int16
EXP = mybir.ActivationFunctionType.Exp

# Schraudolph fast-exp constants (DVE path): exp(s/8 + m) is approximated by
# computing I = round(s*SCH_A + m*SCH_M + SCH_B) as int16 and reinterpreting
# the bits as bf16 (sign|8exp|7mant). Max prob ripple ~±3.3%, which washes
# out to ~2e-3 absmax-relative error in the context output.
_LOG2E = 1.4426950408889634
SCH_A = 16.0 * _LOG2E               # multiplies the raw score (exp(s/8))
SCH_M = 128.0 * _LOG2E              # multiplies the additive mask
SCH_B = 128.0 * (127.0 - 0.0575327) # exponent bias minus zero-mean shift

P = 128
HH = 768          # hidden size
HB = HH // P      # 6 h-blocks
NHEAD = 3         # heads per core
HN = 64
FQKV = 576        # packed feature columns per core
QCHUNK = 512
B, S, H = 2, 4096, 768
N_CORES = 8


def _build(nc: bass.Bass, S: int = S):
    TB = S // P               # token blocks
    QC = S // QCHUNK          # q chunks
    assert QC % 2 == 0

    hs_d = nc.dram_tensor("hs", [HH, S], BF16, kind="ExternalInput").ap()
    w_d = nc.dram_tensor("w", [HH, FQKV], F32, kind="ExternalInput").ap()
    b_d = nc.dram_tensor("b", [640, 1], F32, kind="ExternalInput").ap()
    bflat_d = nc.dram_tensor("bflat", [1, 640], F32, kind="ExternalInput").ap()
    mask_d = nc.dram_tensor("mask", [S, 1], F32, kind="ExternalInput").ap()
    out_d = nc.dram_tensor("out", [S, NHEAD * HN], F32, kind="ExternalOutput").ap()

    with tile.TileContext(nc) as tc, ExitStack() as st_p:
        pool_p = st_p.enter_context(tc.tile_pool(name="persist", bufs=1))

        hT = pool_p.tile([P, HB, S], BF16, tag="hT")
        QT01 = pool_p.tile([P, S], BF16, tag="QT01")
        KT01 = pool_p.tile([P, S], BF16, tag="KT01")
        QT2 = pool_p.tile([P, S], BF16, tag="QT2")
        KT2 = pool_p.tile([P, S], BF16, tag="KT2")
        VZ = pool_p.tile([P, TB, NHEAD, HN + 1], BF16, tag="VZ")
        outsb = pool_p.tile([P, TB, NHEAD * HN], F32, tag="outsb")
        wb = pool_p.tile([P, HB, FQKV], BF16, tag="wb")
        btile = pool_p.tile([P, 5], F32, tag="btile")
        bvrow = pool_p.tile([1, NHEAD * HN], F32, tag="bvrow")
        bvb = pool_p.tile([P, NHEAD, HN], F32, tag="bvb")
        masks = pool_p.tile([P, TB], F32, tag="masks")
        masks2 = pool_p.tile([P, TB], F32, tag="masks2")
        ident = pool_p.tile([P, P], F32, tag="ident")
        identb = pool_p.tile([P, P], BF16, tag="identb")

        make_identity(nc, ident[:])
        nc.vector.tensor_copy(identb[:], ident[:])
        nc.vector.memset(VZ[:, :, :, HN : HN + 1], 1.0)

        # PE warm-up: the first real matmuls are gated on ~10us of input DMA,
        # long enough for the HAM activity monitor to hold the PE at half
        # clock (K=4/8). Feed it dummy identity matmuls that depend only on
        # on-chip data so the array is at 2.4 GHz when the real work arrives.
        with ExitStack() as st_wu:
            pool_wu = st_wu.enter_context(
                tc.tile_pool(name="wu", bufs=1, space="PSUM")
            )
            wups = pool_wu.tile([P, P], F32, tag="wups")
            for _ in range(80):
                nc.tensor.matmul(
                    wups[:], identb[:], identb[:], start=True, stop=True,
                    skip_group_check=True,
                )


        # ---- phase 1+2: load/cast/transpose hidden; QKV projection ----
        with ExitStack() as st_12:
            pool_ld = st_12.enter_context(tc.tile_pool(name="ld", bufs=3))
            pool_qkps = st_12.enter_context(
                tc.tile_pool(name="qkps", bufs=3, space="PSUM")
            )
            pool_vps = st_12.enter_context(
                tc.tile_pool(name="vps", bufs=2, space="PSUM")
            )
            pool_bv = pool_vps

            wf32 = pool_ld.tile([P, HB, FQKV], F32, tag="wf32")
            for hb in range(HB):
                nc.scalar.dma_start(out=wf32[:, hb, :], in_=w_d[ts(hb, P), :])
            nc.vector.tensor_copy(wb[:], wf32[:])

            # small loads on the scalar HWDGE queue (sync queue is for hidden)
            for fb in range(5):
                nc.scalar.dma_start(out=btile[:, fb : fb + 1], in_=b_d[ts(fb, P), :])
            nc.scalar.dma_start(out=bvrow[:], in_=bflat_d[:, 384:576])
            for tb in range(TB):
                nc.scalar.dma_start(
                    out=masks[:, tb : tb + 1], in_=mask_d[ts(tb, P), :]
                )
            # per-token bias for the DVE fast-exp path
            nc.vector.tensor_scalar(
                masks2[:], masks[:], SCH_M, SCH_B,
                op0=mybir.AluOpType.mult, op1=mybir.AluOpType.add,
            )

            # V-bias broadcast row -> [128, 192] via K=1 matmul
            ones1 = pool_ld.tile([1, P], F32, tag="ones1")
            nc.vector.memset(ones1[:], 1.0)
            bvps = pool_bv.tile([P, NHEAD, HN], F32, tag="bvps")
            nc.tensor.matmul(bvps[:], ones1[:], bvrow[:], start=True, stop=True)
            nc.vector.tensor_copy(bvb[:], bvps[:])

            # hidden arrives pre-transposed from the host ([HH, S] bf16), so
            # hT fills with plain contiguous DMAs, chunked in token-quarters
            # so early QKV/V matmuls can start after ~1/4 of the transfer.
            # (Striping across a second queue was measured slower.)
            SQ = S // 4
            for quar in range(4):
                for hb in range(HB):
                    nc.sync.dma_start(
                        out=hT[:, hb, ts(quar, SQ)],
                        in_=hs_d[ts(hb, P), ts(quar, SQ)],
                    )

            # (Splitting these K=128 contractions into row-tiled K=64 halves
            # was measured slower: accumulation-chain matmuls serialize
            # within a row group, so the MM count doubles without overlap.)
            TPQ = QCHUNK // P  # token blocks per chunk
            for tq in range(S // QCHUNK):
                for tbl in range(TPQ):
                    tb = tq * TPQ + tbl
                    # V natural orientation: lhsT = hT blocks, rhs = W_v cols
                    vv = pool_vps.tile([P, NHEAD, HN], F32, tag="vv")
                    for hb in range(HB):
                        nc.tensor.matmul(
                            vv[:],
                            hT[:, hb, ts(tb, P)],
                            wb[:, hb, 384:576],
                            start=(hb == 0),
                            stop=(hb == HB - 1),
                        )
                    nc.vector.tensor_tensor(
                        VZ[:, tb, :, 0:HN], vv[:], bvb[:], op=mybir.AluOpType.add
                    )

                # mixed_T f-blocks (Q0Q1, K0K1, Q2K2) for this token chunk
                for fb in range(3):
                    mm = pool_qkps.tile([P, QCHUNK], F32, tag="mm")
                    for hb in range(HB):
                        nc.tensor.matmul(
                            mm[:],
                            wb[:, hb, ts(fb, P)],
                            hT[:, hb, ts(tq, QCHUNK)],
                            start=(hb == 0),
                            stop=(hb == HB - 1),
                        )
                    dst = ts(tq, QCHUNK)
                    if fb == 0:
                        nc.vector.tensor_scalar_add(
                            QT01[:, dst], mm[:], btile[:, 0:1]
                        )
                    elif fb == 1:
                        nc.vector.tensor_scalar_add(
                            KT01[:, dst], mm[:], btile[:, 1:2]
                        )
                    else:
                        nc.vector.tensor_scalar_add(
                            QT2[0:HN, dst], mm[0:HN, :], btile[0:HN, 2:3]
                        )
                        nc.vector.tensor_scalar_add(
                            KT2[HN:P, dst], mm[HN:P, :], btile[HN:P, 2:3]
                        )
            # duplicate head-2 Q/K to the other partition half
            nc.sync.dma_start(out=QT2[HN:P, :], in_=QT2[0:HN, :])
            nc.sync.dma_start(out=KT2[0:HN, :], in_=KT2[HN:P, :])

        # ---- phase 3: attention ----
        with ExitStack() as st_3:
            # sc 2 bufs + ct 2 bufs fills PSUM; 3-buf sc with single-buf ct
            # was measured slower (boundary stalls + transpose-scratch
            # contention), as was splitting each step's exp across engines.
            pool_sc = st_3.enter_context(tc.tile_pool(name="sc", bufs=2, space="PSUM"))
            pool_ct = st_3.enter_context(tc.tile_pool(name="ct", bufs=2, space="PSUM"))
            pool_tp = pool_ct  # transpose psum rides the just-freed ct slot
            pool_es = st_3.enter_context(tc.tile_pool(name="es", bufs=4))
            pool_cts = st_3.enter_context(tc.tile_pool(name="cts", bufs=2))
            pool_rz = st_3.enter_context(tc.tile_pool(name="rz", bufs=2))

            pend_q = []

            def emit_postprocess(ct, head, qc):
                """Queue postprocess for ct ([65,512] psum = [ctx_T ; Z]) as
                small pieces, drained one per few steps so the DVE never gets
                a burst that backs up the exp pipeline."""
                cell = {}

                def p0():
                    cell["cts"] = pool_cts.tile(
                        [HN + 1, QCHUNK], F32, tag="cts", name="cts"
                    )
                    nc.vector.tensor_copy(cell["cts"][:], ct[:])

                def pj(j):
                    def f():
                        cts = cell["cts"]
                        tp = pool_tp.tile([P, HN + 1], F32, tag="ctA", name="tp")
                        nc.tensor.transpose(
                            tp[:], cts[:, ts(j, P)], ident[0 : HN + 1, 0 : HN + 1]
                        )
                        rz = pool_rz.tile([P, 1], F32, tag="rz", name="rz")
                        nc.vector.reciprocal(rz[:], tp[:, HN : HN + 1])
                        tb_out = qc * (QCHUNK // P) + j
                        nc.vector.tensor_scalar_mul(
                            outsb[:, tb_out, ts(head, HN)], tp[:, 0:HN], rz[:]
                        )

                    return f

                pend_q.append(p0)
                pend_q.extend(pj(j) for j in range(QCHUNK // P))

            def attn_step(tb, kA, qA, kB, qB, ctA, ctB, vA, vB, first, last,
                          use_dve=False):
                """One t-block: row-packed scores pair, exp, two ctx matmuls.

                Whole-step exp engine alternation (use_dve): ScalarE runs exact
                exp; VectorE runs Schraudolph fast-exp (affine + int16 convert
                whose bits are bf16). One producer + one consumer per tile
                keeps the semaphore graph sparse — measured faster than any
                finer-grained split of the softmax between the two engines.
                """
                sc = pool_sc.tile([P, 2, QCHUNK], F32, tag="sc")
                nc.tensor.matmul(sc[:, 0, :], kA, qA, start=True, stop=True)
                nc.tensor.matmul(sc[:, 1, :], kB, qB, start=True, stop=True)
                es = pool_es.tile([P, 2, QCHUNK], BF16, tag="es")
                if use_dve:
                    nc.vector.tensor_scalar(
                        es[:].bitcast(I16), sc[:], SCH_A,
                        masks2[:, tb : tb + 1],
                        op0=mybir.AluOpType.mult, op1=mybir.AluOpType.add,
                    )
                else:
                    nc.scalar.activation(
                        es[:], sc[:], EXP,
                        bias=masks[:, tb : tb + 1], scale=0.125,
                    )
                nc.tensor.matmul(
                    ctA[:], vA, es[:, 0, :], start=first, stop=last,
                    skip_group_check=True,
                )
                nc.tensor.matmul(
                    ctB[:], vB, es[:, 1, :], start=first, stop=last,
                    skip_group_check=True,
                )

            # (pair-half A args, pair-half B args) per phase step; postprocess
            # of the previous step is deferred past the first few attn_steps
            # of the next so the PE queue never stalls ACT at qc boundaries.
            steps = []
            for qc in range(QC):  # heads 0,1 (partition-paired)
                steps.append((QT01, KT01, (0, qc), (1, qc), 0, 1))
            for qcp in range(QC // 2):  # head 2 (self-paired across q-chunks)
                steps.append((QT2, KT2, (2, 2 * qcp), (2, 2 * qcp + 1), 2, 2))

            for QT, KT, (hA, qcA), (hB, qcB), hvA, hvB in steps:
                ctA = pool_ct.tile([HN + 1, QCHUNK], F32, tag="ctA")
                ctB = pool_ct.tile([HN + 1, QCHUNK], F32, tag="ctB")
                for tb in range(TB):
                    attn_step(
                        tb,
                        KT[0:HN, ts(tb, P)], QT[0:HN, ts(qcA, QCHUNK)],
                        KT[HN:P, ts(tb, P)], QT[HN:P, ts(qcB, QCHUNK)],
                        ctA, ctB,
                        VZ[:, tb, hvA, :], VZ[:, tb, hvB, :],
                        tb == 0, tb == TB - 1,
                        use_dve=(tb % 8) in (1, 3, 6),
                    )
                    if pend_q and 2 <= tb and tb % 3 == 2:
                        pend_q.pop(0)()
                emit_postprocess(ctA, hA, qcA)
                emit_postprocess(ctB, hB, qcB)
            while pend_q:
                pend_q.pop(0)()

            for tb in range(TB):
                nc.sync.dma_start(out=out_d[ts(tb, P), :], in_=outsb[:, tb, :])


_NC_CACHE = None


def _get_nc():
    global _NC_CACHE
    if _NC_CACHE is None:
        nc = bacc.Bacc(
            "TRN2", target_bir_lowering=False, debug=False, num_devices=N_CORES
        )
        _build(nc)
        nc.compile()
        _NC_CACHE = nc
    return _NC_CACHE


def _shard_inputs(hidden_states, attention_mask, W_qkv, b_qkv):
    in_maps = []
    for c in range(N_CORES):
        b, hg = c // 4, c % 4
        h0 = 3 * hg
        order = [(0, h0), (0, h0 + 1), (768, h0), (768, h0 + 1),
                 (0, h0 + 2), (768, h0 + 2),
                 (1536, h0), (1536, h0 + 1), (1536, h0 + 2)]
        cols = np.concatenate(
            [np.arange(off + h * HN, off + (h + 1) * HN) for off, h in order]
        )
        w = np.ascontiguousarray(W_qkv[:, cols], dtype=np.float32)
        bv = np.zeros(640, dtype=np.float32)
        bv[:FQKV] = b_qkv[cols]
        in_maps.append(
            {
                "hs": np.ascontiguousarray(hidden_states[b].T).astype(
                    ml_dtypes.bfloat16
                ),
                "w": w,
                "b": bv[:, None].copy(),
                "bflat": bv[None, :].copy(),
                "mask": np.ascontiguousarray(
                    attention_mask[b, 0, 0, :, None], dtype=np.float32
                ),
            }
        )
    return in_maps


def _unshard(results):
    out = np.empty((B, S, H), dtype=np.float32)
    for c, r in enumerate(results):
        b, hg = c // 4, c % 4
        out[b, :, hg * 192 : (hg + 1) * 192] = r["out"]
    return out


def kernel(hidden_states, attention_mask, W_qkv, b_qkv, _trace=False, _tmpdir=None):
    nc = _get_nc()
    in_maps = _shard_inputs(
        np.asarray(hidden_states), np.asarray(attention_mask),
        np.asarray(W_qkv), np.asarray(b_qkv),
    )
    res = run_bass_kernel_spmd(
        nc, in_maps, core_ids=list(range(N_CORES)), trace=_trace, tmpdir=_tmpdir
    )
    out = _unshard(res.results)
    if _trace:
        kernel.last_exec_time_ns = res.exec_time_ns
        kernel.last_results = res
    return out



# revision 50
# speedup vs baseline: 1.0773x; 1.0090x over previous
"""BERT parallel self-attention on 8 Trainium2 NeuronCores (Bass/Tile).

Self-contained: kernel(**inputs) takes the FULL inputs
  hidden_states [2, 4096, 768] f32, attention_mask [2, 1, 1, 4096] f32,
  W_qkv [768, 2304] f32, b_qkv [2304] f32
and returns the FULL context output [2, 4096, 768] f32.

Sharding (Megatron-style tensor-parallel over heads + data-parallel over
batch): core c handles batch c//4, heads 3*(c%4)..3*(c%4)+2. Each core runs
an identical SPMD program on its shard; host gathers the 8 outputs.

Per-core device program:
  1. hidden arrives host-pre-transposed as [768, S] bf16; plain DMA into
     hT (h on partitions), chunked so QKV matmuls start early.
  2. mixed_T[f, t] = W^T hsT (PE, bf16). Host packs W columns
     [Q0|Q1|K0|K1|Q2|K2|V0|V1|V2] so f-block 0 -> Q_T of heads 0,1 stacked at
     partitions 0-63/64-127 (row-pair layout for the 128x128 PE array),
     f-block 1 -> K_T likewise, f-block 2 -> head 2 (duplicated to both
     halves). V is computed in natural [t, f] orientation with an appended
     ones column (softmax denominator rides the ctx matmul).
  3. attention per (q-chunk, t-block): scores_T[t, q] via two row-packed
     K=64 matmuls; exp alternates between ScalarE (exact LUT exp, 5 of 8
     steps; scale=1/8 folds the 1/sqrt(sqrt(hn))^2 norm, bias=mask[t]) and
     VectorE (3 of 8 steps: Schraudolph fast-exp — one fused affine +
     round-to-nearest int16 convert whose bits are read back as bf16;
     ~+-3% prob ripple, ~1.5e-2 absmax-rel on the output). This splits the
     softmax, the throughput bottleneck, across two engines while keeping
     one producer + one consumer per tile (finer splits measured slower
     due to semaphore density). ctx_T[65, q] += [V|1]^T expS in PSUM.
  4. per (head, q-chunk): PE-transpose 128-blocks of [ctx_T|Z] -> [q, 65],
     reciprocal of Z column, scale, store [q, 192] f32. Postprocess is
     drained in small pieces between attention steps to avoid DVE bursts.
"""

from contextlib import ExitStack

import ml_dtypes
import numpy as np

import concourse.bass as bass
import concourse.mybir as mybir
import concourse.tile as tile
from concourse import bacc
from concourse.bass import ts
from concourse.bass_utils import run_bass_kernel_spmd
from concourse.masks import make_identity

F32 = mybir.dt.float32
BF16 = mybir.dt.bfloat16
I16 = mybir.dt.int16
EXP = mybir.ActivationFunctionType.Exp

# Schraudolph fast-exp constants (DVE path): exp(s/8 + m) is approximated by
# computing I = round(s*SCH_A + m*SCH_M + SCH_B) as int16 and reinterpreting
# the bits as bf16 (sign|8exp|7mant). Max prob ripple ~±3.3%, which washes
# out to ~2e-3 absmax-relative error in the context output.
_LOG2E = 1.4426950408889634
SCH_A = 16.0 * _LOG2E               # multiplies the raw score (exp(s/8))
SCH_M = 128.0 * _LOG2E              # multiplies the additive mask
SCH_B = 128.0 * (127.0 - 0.0575327) # exponent bias minus zero-mean shift

P = 128
HH = 768          # hidden size
HB = HH // P      # 6 h-blocks
NHEAD = 3         # heads per core
HN = 64
FQKV = 576        # packed feature columns per core
QCHUNK = 512
B, S, H = 2, 4096, 768
N_CORES = 8


def _build(nc: bass.Bass, S: int = S):
    TB = S // P               # token blocks
    QC = S // QCHUNK          # q chunks
    assert QC % 2 == 0

    hs_d = nc.dram_tensor("hs", [HH, S], BF16, kind="ExternalInput").ap()
    w_d = nc.dram_tensor("w", [HH, FQKV], F32, kind="ExternalInput").ap()
    b_d = nc.dram_tensor("b", [640, 1], F32, kind="ExternalInput").ap()
    bflat_d = nc.dram_tensor("bflat", [1, 640], F32, kind="ExternalInput").ap()
    mask_d = nc.dram_tensor("mask", [S, 1], F32, kind="ExternalInput").ap()
    out_d = nc.dram_tensor("out", [S, NHEAD * HN], F32, kind="ExternalOutput").ap()

    with tile.TileContext(nc) as tc, ExitStack() as st_p:
        pool_p = st_p.enter_context(tc.tile_pool(name="persist", bufs=1))

        hT = pool_p.tile([P, HB, S], BF16, tag="hT")
        QT01 = pool_p.tile([P, S], BF16, tag="QT01")
        KT01 = pool_p.tile([P, S], BF16, tag="KT01")
        QT2 = pool_p.tile([P, S], BF16, tag="QT2")
        KT2 = pool_p.tile([P, S], BF16, tag="KT2")
        VZ = pool_p.tile([P, TB, NHEAD, HN + 1], BF16, tag="VZ")
        outsb = pool_p.tile([P, TB, NHEAD * HN], F32, tag="outsb")
        wb = pool_p.tile([P, HB, FQKV], BF16, tag="wb")
        btile = pool_p.tile([P, 5], F32, tag="btile")
        bvrow = pool_p.tile([1, NHEAD * HN], F32, tag="bvrow")
        bvb = pool_p.tile([P, NHEAD, HN], F32, tag="bvb")
        masks = pool_p.tile([P, TB], F32, tag="masks")
        masks2 = pool_p.tile([P, TB], F32, tag="masks2")
        ident = pool_p.tile([P, P], F32, tag="ident")
        identb = pool_p.tile([P, P], BF16, tag="identb")

        make_identity(nc, ident[:])
        nc.vector.tensor_copy(identb[:], ident[:])
        nc.vector.memset(VZ[:, :, :, HN : HN + 1], 1.0)


        # ---- phase 1+2: load/cast/transpose hidden; QKV projection ----
        with ExitStack() as st_12:
            pool_ld = st_12.enter_context(tc.tile_pool(name="ld", bufs=3))
            pool_qkps = st_12.enter_context(
                tc.tile_pool(name="qkps", bufs=3, space="PSUM")
            )
            pool_vps = st_12.enter_context(
                tc.tile_pool(name="vps", bufs=2, space="PSUM")
            )
            pool_bv = pool_vps

            wf32 = pool_ld.tile([P, HB, FQKV], F32, tag="wf32")
            for hb in range(HB):
                nc.scalar.dma_start(out=wf32[:, hb, :], in_=w_d[ts(hb, P), :])
            nc.vector.tensor_copy(wb[:], wf32[:])

            # small loads on the scalar HWDGE queue (sync queue is for hidden)
            for fb in range(5):
                nc.scalar.dma_start(out=btile[:, fb : fb + 1], in_=b_d[ts(fb, P), :])
            nc.scalar.dma_start(out=bvrow[:], in_=bflat_d[:, 384:576])
            for tb in range(TB):
                nc.scalar.dma_start(
                    out=masks[:, tb : tb + 1], in_=mask_d[ts(tb, P), :]
                )
            # per-token bias for the DVE fast-exp path
            nc.vector.tensor_scalar(
                masks2[:], masks[:], SCH_M, SCH_B,
                op0=mybir.AluOpType.mult, op1=mybir.AluOpType.add,
            )

            # V-bias broadcast row -> [128, 192] via K=1 matmul
            ones1 = pool_ld.tile([1, P], F32, tag="ones1")
            nc.vector.memset(ones1[:], 1.0)
            bvps = pool_bv.tile([P, NHEAD, HN], F32, tag="bvps")
            nc.tensor.matmul(bvps[:], ones1[:], bvrow[:], start=True, stop=True)
            nc.vector.tensor_copy(bvb[:], bvps[:])

            # hidden arrives pre-transposed from the host ([HH, S] bf16), so
            # hT fills with plain contiguous DMAs, chunked in token-quarters
            # so early QKV/V matmuls can start after ~1/4 of the transfer.
            # (Striping across a second queue was measured slower.)
            SQ = S // 4
            for quar in range(4):
                for hb in range(HB):
                    nc.sync.dma_start(
                        out=hT[:, hb, ts(quar, SQ)],
                        in_=hs_d[ts(hb, P), ts(quar, SQ)],
                    )

            # (Splitting these K=128 contractions into row-tiled K=64 halves
            # was measured slower: accumulation-chain matmuls serialize
            # within a row group, so the MM count doubles without overlap.)
            TPQ = QCHUNK // P  # token blocks per chunk
            for tq in range(S // QCHUNK):
                for tbl in range(TPQ):
                    tb = tq * TPQ + tbl
                    # V natural orientation: lhsT = hT blocks, rhs = W_v cols
                    vv = pool_vps.tile([P, NHEAD, HN], F32, tag="vv")
                    for hb in range(HB):
                        nc.tensor.matmul(
                            vv[:],
                            hT[:, hb, ts(tb, P)],
                            wb[:, hb, 384:576],
                            start=(hb == 0),
                            stop=(hb == HB - 1),
                        )
                    nc.vector.tensor_tensor(
                        VZ[:, tb, :, 0:HN], vv[:], bvb[:], op=mybir.AluOpType.add
                    )

                # mixed_T f-blocks (Q0Q1, K0K1, Q2K2) for this token chunk
                for fb in range(3):
                    mm = pool_qkps.tile([P, QCHUNK], F32, tag="mm")
                    for hb in range(HB):
                        nc.tensor.matmul(
                            mm[:],
                            wb[:, hb, ts(fb, P)],
                            hT[:, hb, ts(tq, QCHUNK)],
                            start=(hb == 0),
                            stop=(hb == HB - 1),
                        )
                    dst = ts(tq, QCHUNK)
                    if fb == 0:
                        nc.vector.tensor_scalar_add(
                            QT01[:, dst], mm[:], btile[:, 0:1]
                        )
                    elif fb == 1:
                        nc.vector.tensor_scalar_add(
                            KT01[:, dst], mm[:], btile[:, 1:2]
                        )
                    else:
                        nc.vector.tensor_scalar_add(
                            QT2[0:HN, dst], mm[0:HN, :], btile[0:HN, 2:3]
                        )
                        nc.vector.tensor_scalar_add(
                            KT2[HN:P, dst], mm[HN:P, :], btile[HN:P, 2:3]
                        )
            # duplicate head-2 Q/K to the other partition half
            nc.sync.dma_start(out=QT2[HN:P, :], in_=QT2[0:HN, :])
            nc.sync.dma_start(out=KT2[0:HN, :], in_=KT2[HN:P, :])

        # ---- phase 3: attention ----
        with ExitStack() as st_3:
            # sc 2 bufs + ct 2 bufs fills PSUM; 3-buf sc with single-buf ct
            # was measured slower (boundary stalls + transpose-scratch
            # contention), as was splitting each step's exp across engines.
            pool_sc = st_3.enter_context(tc.tile_pool(name="sc", bufs=2, space="PSUM"))
            pool_ct = st_3.enter_context(tc.tile_pool(name="ct", bufs=2, space="PSUM"))
            pool_tp = pool_ct  # transpose psum rides the just-freed ct slot
            pool_es = st_3.enter_context(tc.tile_pool(name="es", bufs=4))
            pool_cts = st_3.enter_context(tc.tile_pool(name="cts", bufs=2))
            pool_rz = st_3.enter_context(tc.tile_pool(name="rz", bufs=2))

            pend_q = []

            def emit_postprocess(ct, head, qc):
                """Queue postprocess for ct ([65,512] psum = [ctx_T ; Z]) as
                small pieces, drained one per few steps so the DVE never gets
                a burst that backs up the exp pipeline."""
                cell = {}

                def p0():
                    cell["cts"] = pool_cts.tile(
                        [HN + 1, QCHUNK], F32, tag="cts", name="cts"
                    )
                    nc.vector.tensor_copy(cell["cts"][:], ct[:])

                def pj(j):
                    def f():
                        cts = cell["cts"]
                        tp = pool_tp.tile([P, HN + 1], F32, tag="ctA", name="tp")
                        nc.tensor.transpose(
                            tp[:], cts[:, ts(j, P)], ident[0 : HN + 1, 0 : HN + 1]
                        )
                        rz = pool_rz.tile([P, 1], F32, tag="rz", name="rz")
                        nc.vector.reciprocal(rz[:], tp[:, HN : HN + 1])
                        tb_out = qc * (QCHUNK // P) + j
                        nc.vector.tensor_scalar_mul(
                            outsb[:, tb_out, ts(head, HN)], tp[:, 0:HN], rz[:]
                        )

                    return f

                pend_q.append(p0)
                pend_q.extend(pj(j) for j in range(QCHUNK // P))

            def attn_step(tb, kA, qA, kB, qB, ctA, ctB, vA, vB, first, last,
                          use_dve=False):
                """One t-block: row-packed scores pair, exp, two ctx matmuls.

                Whole-step exp engine alternation (use_dve): ScalarE runs exact
                exp; VectorE runs Schraudolph fast-exp (affine + int16 convert
                whose bits are bf16). One producer + one consumer per tile
                keeps the semaphore graph sparse — measured faster than any
                finer-grained split of the softmax between the two engines.
                """
                sc = pool_sc.tile([P, 2, QCHUNK], F32, tag="sc")
                nc.tensor.matmul(sc[:, 0, :], kA, qA, start=True, stop=True)
                nc.tensor.matmul(sc[:, 1, :], kB, qB, start=True, stop=True)
                es = pool_es.tile([P, 2, QCHUNK], BF16, tag="es")
                if use_dve:
                    nc.vector.tensor_scalar(
                        es[:].bitcast(I16), sc[:], SCH_A,
                        masks2[:, tb : tb + 1],
                        op0=mybir.AluOpType.mult, op1=mybir.AluOpType.add,
                    )
                else:
                    nc.scalar.activation(
                        es[:], sc[:], EXP,
                        bias=masks[:, tb : tb + 1], scale=0.125,
                    )
                nc.tensor.matmul(
                    ctA[:], vA, es[:, 0, :], start=first, stop=last,
                    skip_group_check=True,
                )
                nc.tensor.matmul(
                    ctB[:], vB, es[:, 1, :], start=first, stop=last,
                    skip_group_check=True,
                )

            # (pair-half A args, pair-half B args) per phase step; postprocess
            # of the previous step is deferred past the first few attn_steps
            # of the next so the PE queue never stalls ACT at qc boundaries.
            steps = []
            for qc in range(QC):  # heads 0,1 (partition-paired)
                steps.append((QT01, KT01, (0, qc), (1, qc), 0, 1))
            for qcp in range(QC // 2):  # head 2 (self-paired across q-chunks)
                steps.append((QT2, KT2, (2, 2 * qcp), (2, 2 * qcp + 1), 2, 2))

            for QT, KT, (hA, qcA), (hB, qcB), hvA, hvB in steps:
                ctA = pool_ct.tile([HN + 1, QCHUNK], F32, tag="ctA")
                ctB = pool_ct.tile([HN + 1, QCHUNK], F32, tag="ctB")
                for tb in range(TB):
                    attn_step(
                        tb,
                        KT[0:HN, ts(tb, P)], QT[0:HN, ts(qcA, QCHUNK)],
                        KT[HN:P, ts(tb, P)], QT[HN:P, ts(qcB, QCHUNK)],
                        ctA, ctB,
                        VZ[:, tb, hvA, :], VZ[:, tb, hvB, :],
                        tb == 0, tb == TB - 1,
                        use_dve=(tb % 8) in (1, 3, 6),
                    )
                    if pend_q and 2 <= tb and tb % 3 == 2:
                        pend_q.pop(0)()
                emit_postprocess(ctA, hA, qcA)
                emit_postprocess(ctB, hB, qcB)
            while pend_q:
                pend_q.pop(0)()

            # stripe the output store across both HWDGE queues so the tail
            # (gated on the last postprocess pieces) drains twice as fast
            for tb in range(TB):
                q_eng = nc.sync if tb % 2 == 0 else nc.scalar
                q_eng.dma_start(out=out_d[ts(tb, P), :], in_=outsb[:, tb, :])


_NC_CACHE = None


def _get_nc():
    global _NC_CACHE
    if _NC_CACHE is None:
        nc = bacc.Bacc(
            "TRN2", target_bir_lowering=False, debug=False, num_devices=N_CORES
        )
        _build(nc)
        nc.compile()
        _NC_CACHE = nc
    return _NC_CACHE


def _shard_inputs(hidden_states, attention_mask, W_qkv, b_qkv):
    in_maps = []
    for c in range(N_CORES):
        b, hg = c // 4, c % 4
        h0 = 3 * hg
        order = [(0, h0), (0, h0 + 1), (768, h0), (768, h0 + 1),
                 (0, h0 + 2), (768, h0 + 2),
                 (1536, h0), (1536, h0 + 1), (1536, h0 + 2)]
        cols = np.concatenate(
            [np.arange(off + h * HN, off + (h + 1) * HN) for off, h in order]
        )
        w = np.ascontiguousarray(W_qkv[:, cols], dtype=np.float32)
        bv = np.zeros(640, dtype=np.float32)
        bv[:FQKV] = b_qkv[cols]
        in_maps.append(
            {
                "hs": np.ascontiguousarray(hidden_states[b].T).astype(
                    ml_dtypes.bfloat16
                ),
                "w": w,
                "b": bv[:, None].copy(),
                "bflat": bv[None, :].copy(),
                "mask": np.ascontiguousarray(
                    attention_mask[b, 0, 0, :, None], dtype=np.float32
                ),
            }
        )
    return in_maps


def _unshard(results):
    out = np.empty((B, S, H), dtype=np.float32)
    for c, r in enumerate(results):
        b, hg = c // 4, c % 4
        out[b, :, hg * 192 : (hg + 1) * 192] = r["out"]
    return out


def kernel(hidden_states, attention_mask, W_qkv, b_qkv, _trace=False, _tmpdir=None):
    nc = _get_nc()
    in_maps = _shard_inputs(
        np.asarray(hidden_states), np.asarray(attention_mask),
        np.asarray(W_qkv), np.asarray(b_qkv),
    )
    res = run_bass_kernel_spmd(
        nc, in_maps, core_ids=list(range(N_CORES)), trace=_trace, tmpdir=_tmpdir
    )
    out = _unshard(res.results)
    if _trace:
        kernel.last_exec_time_ns = res.exec_time_ns
        kernel.last_results = res
    return out

